# revision 15
# baseline (speedup 1.0000x reference)
"""Causal sliding-window attention (window=256, temperature=8) on Trainium2.

Problem: q,k,v [B=2, H=16, S=2048, D=64] f32.  Returns (out, attn) like the
reference: out = softmax(mask(QK^T/8)) @ V and attn = the full [S, S]
probability matrix (exactly zero outside the causal 256-wide band).

Sharding: the 32 (b,h) pairs are data-parallel; each of the 8 NeuronCores
processes 4 heads end-to-end (no cross-core communication).

Per head on-device (transposed-scores dataflow):
  - PE-transpose q,k into [D, S] layout (contraction on partitions).
  - per 128-row query tile t the allowed keys are block columns
    [t-2, t-1, t].  Each S^T block [keys=128, q=128] is one matmul
    (lhsT = kT block, rhs = qT tile); the 3 blocks land in one PSUM bank.
  - one Exp activation (temperature folded into the scale) moves all
    3 blocks PSUM -> SBUF; the GPSIMD engine applies the multiplicative
    0/1 band masks (exact zeros, matching the reference's -1e9 underflow).
  - the masked exp tile IS P^T (unnormalized): it is DMA'd out compactly
    as the band, and feeds P@V as matmul(lhsT = v_ext, rhs = P^T block)
    where v_ext carries an extra all-ones column 64 -> the accumulated
    [65, q] PSUM holds both out^T (rows 0..63) and the softmax
    denominators (row 64), all computed on device.
Host side: transposes/scatters the band blocks into the zero [S, S]
matrix and divides by the device-computed denominators.
"""

import json

import numpy as np

import concourse.bass as bass
import concourse.bass2jax as bass2jax
import concourse.bass_utils as bass_utils
import concourse.mybir as mybir
import concourse.tile as tile
from concourse.bass import ts
from concourse.bass_utils import run_bass_kernel_spmd
from concourse.masks import make_identity

# ---------------------------------------------------------------------------
# Wait legalization: the walrus build in this container accepts only ONE
# sync wait per instruction (setupSyncWait: "Too many sync wait commands"),
# but Tile's semaphore assignment freely attaches several.  Split every
# excess wait into a standalone single-wait EventSemaphore instruction on
# the same engine immediately before the real instruction (the engine's
# sequencer blocks on each in turn — semantics preserved).
# ---------------------------------------------------------------------------

_MAX_WAITS = 1


def _legalize_block(bb, counter):
    out_instrs = []
    for ins in bb.get("instructions", []):
        si = ins.get("sync_info")
        waits = (si or {}).get("on_wait") or []
        if len(waits) > _MAX_WAITS:
            keep = waits[-_MAX_WAITS:]
            hoist = waits[:-_MAX_WAITS]
            for w in hoist:
                counter[0] += 1
                out_instrs.append({
                    "debug": ins.get("debug", 0),
                    "engine": ins["engine"],
                    "ins": [],
                    "name": f"evw_{counter[0]}_{ins['name']}",
                    "opcode": "EventSemaphore",
                    "outs": [],
                    "sync_info": {"on_update": [], "on_wait": [w]},
                })
            si["on_wait"] = keep
        out_instrs.append(ins)
    bb["instructions"] = out_instrs
    for sub in bb.get("blocks", []):
        _legalize_block(sub, counter)


def _legalize_waits(bir_json):
    d = json.loads(bir_json)
    counter = [0]
    for f in d.get("functions", []):
        for bb in f.get("blocks", []):
            _legalize_block(bb, counter)
    return json.dumps(d).encode()


_orig_compile_bir_kernel = bass_utils.compile_bir_kernel


def _patched_compile_bir_kernel(bir_json, tmpdir, neff_name="file.neff"):
    return _orig_compile_bir_kernel(_legalize_waits(bir_json), tmpdir,
                                    neff_name=neff_name)


if getattr(bass_utils.compile_bir_kernel, "__name__", "") != "_patched_compile_bir_kernel":
    bass_utils.compile_bir_kernel = _patched_compile_bir_kernel
    bass2jax.compile_bir_kernel = _patched_compile_bir_kernel


F32 = mybir.dt.float32
P = 128          # partition / query tile rows
D = 64           # head dim
DE = D + 1       # head dim + ones column (denominator row)
S = 2048         # sequence length
T = S // P       # 16 query tiles per head
W = 3 * P        # widest key window per query tile (2 prev blocks + diag)
NH = 4           # (b, h) pairs per core
NCORES = 8
TEMP = 8.0


def _build(repeat=1, _noband=False):
    nc = bass.Bass("TRN2", target_bir_lowering=False, debug=False)
    q = nc.dram_tensor("q", [NH, S, D], F32, kind="ExternalInput").ap()
    k = nc.dram_tensor("k", [NH, S, D], F32, kind="ExternalInput").ap()
    v = nc.dram_tensor("v", [NH, S, D], F32, kind="ExternalInput").ap()
    # out^T per tile with the denominator row: [hd, t, 65, q]
    out = nc.dram_tensor("out", [NH, T, DE, P], F32, kind="ExternalOutput").ap()
    # band[hd, t, kk, c*128+q] = P^T_unnorm block c of query tile t
    band = nc.dram_tensor("band", [NH, T, P, W], F32, kind="ExternalOutput").ap()

    with tile.TileContext(nc) as tc:
        with (
            tc.tile_pool(name="consts", bufs=1) as consts,
            tc.tile_pool(name="perhead", bufs=2) as perhead,
            tc.tile_pool(name="work", bufs=5) as work,
            tc.tile_pool(name="ps_tr", bufs=2, space="PSUM") as ps_tr,
            tc.tile_pool(name="ps_st", bufs=3, space="PSUM") as ps_st,
            tc.tile_pool(name="ps_o", bufs=2, space="PSUM") as ps_o,
        ):
            ident = consts.tile([P, P], F32)
            make_identity(nc, ident)

            # Multiplicative masks for S^T blocks [key jj (partition), q r]:
            # m2[:, 0, :]: diagonal block (j=0), allowed iff jj <= r.
            # m2[:, 1, :]: t-2 block (j=2),      allowed iff jj > r.
            m2 = consts.tile([P, 2, P], F32)
            nc.gpsimd.memset(m2, 1.0)
            nc.gpsimd.affine_select(
                m2[:, 0, :], m2[:, 0, :],
                compare_op=mybir.AluOpType.is_ge,
                fill=0.0, base=0, channel_multiplier=-1, pattern=[[1, P]],
            )
            nc.gpsimd.affine_select(
                m2[:, 1, :], m2[:, 1, :],
                compare_op=mybir.AluOpType.is_gt,
                fill=0.0, base=0, channel_multiplier=1, pattern=[[-1, P]],
            )

            def emit_head(hd):
                q_nat = perhead.tile([P, T, D], F32, tag="qnat")
                k_nat = perhead.tile([P, T, D], F32, tag="knat")
                v_sb = perhead.tile([P, T, DE], F32, tag="v")
                nc.sync.dma_start(q_nat, q[hd].rearrange("(t p) d -> p t d", p=P))
                nc.sync.dma_start(k_nat, k[hd].rearrange("(t p) d -> p t d", p=P))
                nc.sync.dma_start(v_sb[:, :, 0:D],
                                  v[hd].rearrange("(t p) d -> p t d", p=P))
                nc.gpsimd.memset(v_sb[:, :, D:DE], 1.0)

                # q, k transposed to [D, t, 128] (contraction dim on partitions)
                qT = perhead.tile([D, T, P], F32, tag="qT")
                kT = perhead.tile([D, T, P], F32, tag="kT")
                for t4 in range(T // 4):
                    tq = ps_tr.tile([D, 4, P], F32, tag="tqk")
                    tk = ps_tr.tile([D, 4, P], F32, tag="tqk")
                    for j in range(4):
                        t = 4 * t4 + j
                        nc.tensor.transpose(tq[:, j], q_nat[:, t, :], ident)
                        nc.tensor.transpose(tk[:, j], k_nat[:, t, :], ident)
                    nc.scalar.copy(qT[:, ts(t4, 4), :], tq)
                    nc.vector.tensor_copy(kT[:, ts(t4, 4), :], tk)

                oT_all = perhead.tile([DE, T, P], F32, tag="o")

                # Iteration c produces P^T block c (vs query tiles c..c+2) and
                # consumes blocks c-2..c for query tile t = c's P @ V.
                o4 = None
                pTb = [None] * T
                for c in range(T):
                    jn = min(3, T - c)      # query tiles covered by block c

                    stp = ps_st.tile([P, 3, P], F32, tag="st")
                    nc.tensor.matmul(
                        stp[:, :jn, :], kT[:, c, :], qT[:, c:c + jn, :],
                        start=True, stop=True,
                    )

                    pT = work.tile([P, 3, P], F32, tag="pT")
                    pTb[c] = pT
                    nc.scalar.activation(
                        pT[:, :jn, :], stp[:, :jn, :],
                        mybir.ActivationFunctionType.Exp, scale=1.0 / TEMP,
                    )
                    # band masks (exact 0 for disallowed entries): j=0 is the
                    # diagonal block of tile c, j=2 the t-2 block of tile c+2
                    if jn == 3:
                        nc.gpsimd.tensor_mul(pT[:, 0::2, :], pT[:, 0::2, :], m2)
                    else:
                        nc.gpsimd.tensor_mul(pT[:, 0, :], pT[:, 0, :],
                                             m2[:, 0, :])

                    if not _noband:
                        nc.sync.dma_start(band[hd, c, :, :jn * P], pT[:, :jn, :])

                    # P @ V for query tile t = c (its window is complete now)
                    t = c
                    w0b = max(0, t - 2)
                    if t % 4 == 0:
                        o4 = ps_o.tile([DE, 4, P], F32, tag="o4")
                    for c2 in range(w0b, t + 1):
                        nc.tensor.matmul(
                            o4[:, t % 4, :], v_sb[:, c2, :],
                            pTb[c2][:, t - c2, :],
                            start=(c2 == w0b), stop=(c2 == t),
                        )
                    if t % 4 == 3:
                        nc.vector.tensor_copy(oT_all[:, t - 3:t + 1, :], o4)

                nc.sync.dma_start(out[hd].rearrange("t e q -> e t q"), oT_all)

            def emit_heads():
                for hd in range(NH):
                    emit_head(hd)

            if repeat == 1:
                emit_heads()
            else:
                with tc.For_i(0, repeat, 1):
                    emit_heads()
    return nc


LAST_RESULTS = None


def _run(in_maps, trace=False):
    global LAST_RESULTS
    nc = _build()
    LAST_RESULTS = run_bass_kernel_spmd(
        nc, in_maps, core_ids=list(range(NCORES)), trace=trace,
    )
    return LAST_RESULTS.results


def kernel(q, k, v, _trace=False):
    q = np.ascontiguousarray(np.asarray(q, dtype=np.float32))
    k = np.ascontiguousarray(np.asarray(k, dtype=np.float32))
    v = np.ascontiguousarray(np.asarray(v, dtype=np.float32))
    B, H, S_, D_ = q.shape
    assert (S_, D_) == (S, D), (S_, D_)
    G = B * H
    per = G // NCORES
    assert per == NH

    qf = q.reshape(G, S, D)
    kf = k.reshape(G, S, D)
    vf = v.reshape(G, S, D)
    in_maps = [
        {
            "q": np.ascontiguousarray(qf[i * per:(i + 1) * per]),
            "k": np.ascontiguousarray(kf[i * per:(i + 1) * per]),
            "v": np.ascontiguousarray(vf[i * per:(i + 1) * per]),
        }
        for i in range(NCORES)
    ]

    results = _run(in_maps, trace=_trace)

    out = np.empty((G, S, D), np.float32)
    attn = np.zeros((G, S, S), np.float32)
    for i in range(NCORES):
        outr = results[i]["out"]                       # [NH, T, DE, P]
        bandr = results[i]["band"]                     # [NH, C=T, P, W]
        for g in range(per):
            gi = i * per + g
            den = outr[g, :, D, :]                     # [T, P] per-query sums
            ot = outr[g, :, :D, :] / den[:, None, :]   # [T, D, P]
            out[gi] = ot.transpose(0, 2, 1).reshape(S, D)
            # band[g, c, kk, j*128+q] -> attn[128(c+j)+q, 128c+kk]
            bt = bandr[g].reshape(T, P, 3, P)          # [c, kk, j, q]
            bt = bt.transpose(0, 2, 3, 1)              # [c, j, q, kk]
            for c in range(T):
                for j in range(min(3, T - c)):
                    t = c + j
                    attn[gi, t * P:(t + 1) * P, c * P:(c + 1) * P] = (
                        bt[c, j] / den[t][:, None]
                    )
    return out.reshape(B, H, S, D), attn.reshape(B, H, S, S)


# revision 16
# speedup vs baseline: 1.0179x; 1.0179x over previous
"""Causal sliding-window attention (window=256, temperature=8) on Trainium2.

Problem: q,k,v [B=2, H=16, S=2048, D=64] f32.  Returns (out, attn) like the
reference: out = softmax(mask(QK^T/8)) @ V and attn = the full [S, S]
probability matrix (exactly zero outside the causal 256-wide band).

Sharding: the 32 (b,h) pairs are data-parallel; each of the 8 NeuronCores
processes 4 heads end-to-end (no cross-core communication).

Per head on-device (transposed-scores dataflow):
  - PE-transpose q,k into [D, S] layout (contraction on partitions).
  - per 128-key block c, one matmul (lhsT = kT block, rhs = qT tiles
    c..c+2) produces S^T block c against the (up to) 3 query tiles whose
    window contains it, in one PSUM bank [128, 3, 128].
  - one Exp activation (temperature folded into the scale) moves all
    3 blocks PSUM -> SBUF; the GPSIMD engine applies the multiplicative
    0/1 band masks (exact zeros, matching the reference's -1e9 underflow).
  - the masked exp tile IS P^T (unnormalized): it is DMA'd out compactly
    as the band, and feeds P@V as matmul(lhsT = v_ext, rhs = P^T block)
    where v_ext carries an extra all-ones column 64 -> the accumulated
    [65, q] PSUM holds both out^T (rows 0..63) and the softmax
    denominators (row 64), all computed on device.
Host side: transposes/scatters the band blocks into the zero [S, S]
matrix and divides by the device-computed denominators.
"""

import json

import numpy as np

import concourse.bass as bass
import concourse.bass2jax as bass2jax
import concourse.bass_utils as bass_utils
import concourse.mybir as mybir
import concourse.tile as tile
from concourse.bass import ts
from concourse.bass_utils import run_bass_kernel_spmd
from concourse.masks import make_identity

# ---------------------------------------------------------------------------
# Wait legalization: the walrus build in this container accepts only ONE
# sync wait per instruction (setupSyncWait: "Too many sync wait commands"),
# but Tile's semaphore assignment freely attaches several.  Split every
# excess wait into a standalone single-wait EventSemaphore instruction on
# the same engine immediately before the real instruction (the engine's
# sequencer blocks on each in turn — semantics preserved).
# ---------------------------------------------------------------------------

_MAX_WAITS = 1


def _legalize_block(bb, counter):
    out_instrs = []
    for ins in bb.get("instructions", []):
        si = ins.get("sync_info")
        waits = (si or {}).get("on_wait") or []
        if len(waits) > _MAX_WAITS:
            keep = waits[-_MAX_WAITS:]
            hoist = waits[:-_MAX_WAITS]
            for w in hoist:
                counter[0] += 1
                out_instrs.append({
                    "debug": ins.get("debug", 0),
                    "engine": ins["engine"],
                    "ins": [],
                    "name": f"evw_{counter[0]}_{ins['name']}",
                    "opcode": "EventSemaphore",
                    "outs": [],
                    "sync_info": {"on_update": [], "on_wait": [w]},
                })
            si["on_wait"] = keep
        out_instrs.append(ins)
    bb["instructions"] = out_instrs
    for sub in bb.get("blocks", []):
        _legalize_block(sub, counter)


def _legalize_waits(bir_json):
    d = json.loads(bir_json)
    counter = [0]
    for f in d.get("functions", []):
        for bb in f.get("blocks", []):
            _legalize_block(bb, counter)
    return json.dumps(d).encode()


_orig_compile_bir_kernel = bass_utils.compile_bir_kernel


def _patched_compile_bir_kernel(bir_json, tmpdir, neff_name="file.neff"):
    return _orig_compile_bir_kernel(_legalize_waits(bir_json), tmpdir,
                                    neff_name=neff_name)


if getattr(bass_utils.compile_bir_kernel, "__name__", "") != "_patched_compile_bir_kernel":
    bass_utils.compile_bir_kernel = _patched_compile_bir_kernel
    bass2jax.compile_bir_kernel = _patched_compile_bir_kernel


F32 = mybir.dt.float32
P = 128          # partition / query tile rows
D = 64           # head dim
DE = D + 1       # head dim + ones column (denominator row)
S = 2048         # sequence length
T = S // P       # 16 query tiles per head
W = 3 * P        # widest key window per query tile (2 prev blocks + diag)
NH = 4           # (b, h) pairs per core
NCORES = 8
TEMP = 8.0


def _build(repeat=1, _noband=False):
    nc = bass.Bass("TRN2", target_bir_lowering=False, debug=False)
    q = nc.dram_tensor("q", [NH, S, D], F32, kind="ExternalInput").ap()
    k = nc.dram_tensor("k", [NH, S, D], F32, kind="ExternalInput").ap()
    v = nc.dram_tensor("v", [NH, S, D], F32, kind="ExternalInput").ap()
    # out^T per tile with the denominator row: [hd, t, 65, q]
    out = nc.dram_tensor("out", [NH, T, DE, P], F32, kind="ExternalOutput").ap()
    # band[hd, t, kk, c*128+q] = P^T_unnorm block c of query tile t
    band = nc.dram_tensor("band", [NH, T, P, W], F32, kind="ExternalOutput").ap()

    with tile.TileContext(nc) as tc:
        with (
            tc.tile_pool(name="consts", bufs=1) as consts,
            tc.tile_pool(name="perhead", bufs=2) as perhead,
            tc.tile_pool(name="work", bufs=5) as work,
            tc.tile_pool(name="ps_tr", bufs=2, space="PSUM") as ps_tr,
            tc.tile_pool(name="ps_st", bufs=2, space="PSUM") as ps_st,
            tc.tile_pool(name="ps_o", bufs=2, space="PSUM") as ps_o,
        ):
            ident = consts.tile([P, P], F32)
            make_identity(nc, ident)

            # Multiplicative masks for S^T blocks [key jj (partition), q r]:
            # m2[:, 0, :]: diagonal block (j=0), allowed iff jj <= r.
            # m2[:, 1, :]: t-2 block (j=2),      allowed iff jj > r.
            m2 = consts.tile([P, 2, P], F32)
            nc.gpsimd.memset(m2, 1.0)
            nc.gpsimd.affine_select(
                m2[:, 0, :], m2[:, 0, :],
                compare_op=mybir.AluOpType.is_ge,
                fill=0.0, base=0, channel_multiplier=-1, pattern=[[1, P]],
            )
            nc.gpsimd.affine_select(
                m2[:, 1, :], m2[:, 1, :],
                compare_op=mybir.AluOpType.is_gt,
                fill=0.0, base=0, channel_multiplier=1, pattern=[[-1, P]],
            )

            def emit_head(hd):
                q_nat = perhead.tile([P, T, D], F32, tag="qnat")
                k_nat = perhead.tile([P, T, D], F32, tag="knat")
                v_sb = perhead.tile([P, T, DE], F32, tag="v")
                nc.sync.dma_start(q_nat, q[hd].rearrange("(t p) d -> p t d", p=P))
                nc.sync.dma_start(k_nat, k[hd].rearrange("(t p) d -> p t d", p=P))
                nc.sync.dma_start(v_sb[:, :, 0:D],
                                  v[hd].rearrange("(t p) d -> p t d", p=P))
                nc.gpsimd.memset(v_sb[:, :, D:DE], 1.0)

                # q, k transposed to [D, t, 128] (contraction dim on partitions)
                qT = perhead.tile([D, T, P], F32, tag="qT")
                kT = perhead.tile([D, T, P], F32, tag="kT")
                for t4 in range(T // 4):
                    tq = ps_tr.tile([D, 4, P], F32, tag="tqk")
                    tk = ps_tr.tile([D, 4, P], F32, tag="tqk")
                    for j in range(4):
                        t = 4 * t4 + j
                        nc.tensor.transpose(tq[:, j], q_nat[:, t, :], ident)
                        nc.tensor.transpose(tk[:, j], k_nat[:, t, :], ident)
                    nc.scalar.copy(qT[:, ts(t4, 4), :], tq)
                    nc.vector.tensor_copy(kT[:, ts(t4, 4), :], tk)

                oT_all = perhead.tile([DE, T, P], F32, tag="o")

                # Iteration c produces P^T block c (vs query tiles c..c+2) and
                # consumes blocks c-2..c for query tile t = c's P @ V.
                o4 = None
                pTb = [None] * T
                for c in range(T):
                    jn = min(3, T - c)      # query tiles covered by block c

                    stp = ps_st.tile([P, 3, P], F32, tag="st")
                    nc.tensor.matmul(
                        stp[:, :jn, :], kT[:, c, :], qT[:, c:c + jn, :],
                        start=True, stop=True,
                    )

                    pT = work.tile([P, 3, P], F32, tag="pT")
                    pTb[c] = pT
                    nc.scalar.activation(
                        pT[:, :jn, :], stp[:, :jn, :],
                        mybir.ActivationFunctionType.Exp, scale=1.0 / TEMP,
                    )
                    # band masks (exact 0 for disallowed entries): j=0 is the
                    # diagonal block of tile c, j=2 the t-2 block of tile c+2
                    if jn == 3:
                        nc.gpsimd.tensor_mul(pT[:, 0::2, :], pT[:, 0::2, :], m2)
                    else:
                        nc.gpsimd.tensor_mul(pT[:, 0, :], pT[:, 0, :],
                                             m2[:, 0, :])

                    if not _noband:
                        nc.sync.dma_start(band[hd, c, :, :jn * P], pT[:, :jn, :])

                    # P @ V for query tile t = c (its window is complete now)
                    t = c
                    w0b = max(0, t - 2)
                    if t % 4 == 0:
                        o4 = ps_o.tile([DE, 4, P], F32, tag="o4")
                    for c2 in range(w0b, t + 1):
                        nc.tensor.matmul(
                            o4[:, t % 4, :], v_sb[:, c2, :],
                            pTb[c2][:, t - c2, :],
                            start=(c2 == w0b), stop=(c2 == t),
                        )
                    if t % 4 == 3:
                        nc.vector.tensor_copy(oT_all[:, t - 3:t + 1, :], o4)

                nc.sync.dma_start(out[hd].rearrange("t e q -> e t q"), oT_all)

            def emit_heads():
                for hd in range(NH):
                    emit_head(hd)

            if repeat == 1:
                emit_heads()
            else:
                with tc.For_i(0, repeat, 1):
                    emit_heads()
    return nc


LAST_RESULTS = None


def _run(in_maps, trace=False):
    global LAST_RESULTS
    nc = _build()
    LAST_RESULTS = run_bass_kernel_spmd(
        nc, in_maps, core_ids=list(range(NCORES)), trace=trace,
    )
    return LAST_RESULTS.results


def kernel(q, k, v, _trace=False):
    q = np.ascontiguousarray(np.asarray(q, dtype=np.float32))
    k = np.ascontiguousarray(np.asarray(k, dtype=np.float32))
    v = np.ascontiguousarray(np.asarray(v, dtype=np.float32))
    B, H, S_, D_ = q.shape
    assert (S_, D_) == (S, D), (S_, D_)
    G = B * H
    per = G // NCORES
    assert per == NH

    qf = q.reshape(G, S, D)
    kf = k.reshape(G, S, D)
    vf = v.reshape(G, S, D)
    in_maps = [
        {
            "q": np.ascontiguousarray(qf[i * per:(i + 1) * per]),
            "k": np.ascontiguousarray(kf[i * per:(i + 1) * per]),
            "v": np.ascontiguousarray(vf[i * per:(i + 1) * per]),
        }
        for i in range(NCORES)
    ]

    results = _run(in_maps, trace=_trace)

    out = np.empty((G, S, D), np.float32)
    attn = np.zeros((G, S, S), np.float32)
    for i in range(NCORES):
        outr = results[i]["out"]                       # [NH, T, DE, P]
        bandr = results[i]["band"]                     # [NH, C=T, P, W]
        for g in range(per):
            gi = i * per + g
            den = outr[g, :, D, :]                     # [T, P] per-query sums
            ot = outr[g, :, :D, :] / den[:, None, :]   # [T, D, P]
            out[gi] = ot.transpose(0, 2, 1).reshape(S, D)
            # band[g, c, kk, j*128+q] -> attn[128(c+j)+q, 128c+kk]
            bt = bandr[g].reshape(T, P, 3, P)          # [c, kk, j, q]
            bt = bt.transpose(0, 2, 3, 1)              # [c, j, q, kk]
            for c in range(T):
                for j in range(min(3, T - c)):
                    t = c + j
                    attn[gi, t * P:(t + 1) * P, c * P:(c + 1) * P] = (
                        bt[c, j] / den[t][:, None]
                    )
    return out.reshape(B, H, S, D), attn.reshape(B, H, S, S)


# revision 19
# speedup vs baseline: 1.1386x; 1.1186x over previous
"""Causal sliding-window attention (window=256, temperature=8) on Trainium2.

Problem: q,k,v [B=2, H=16, S=2048, D=64] f32.  Returns (out, attn) like the
reference: out = softmax(mask(QK^T/8)) @ V and attn = the full [S, S]
probability matrix (exactly zero outside the causal 256-wide band).

Sharding: the 32 (b,h) pairs are data-parallel; each of the 8 NeuronCores
processes 4 heads end-to-end (no cross-core communication).

Per head on-device (transposed-scores dataflow):
  - PE-transpose q,k into [D, S] layout (contraction on partitions).
  - per 128-key block c, one matmul (lhsT = kT block, rhs = qT tiles
    c..c+2) produces S^T block c against the (up to) 3 query tiles whose
    window contains it, in one PSUM bank [128, 3, 128].
  - one Exp activation (temperature folded into the scale) moves all
    3 blocks PSUM -> SBUF; the GPSIMD engine applies the multiplicative
    0/1 band masks (exact zeros, matching the reference's -1e9 underflow).
  - the masked exp tile IS P^T (unnormalized): it is DMA'd out compactly
    as the band, and feeds P@V as matmul(lhsT = v_ext, rhs = P^T block)
    where v_ext carries an extra all-ones column 64 -> the accumulated
    [65, q] PSUM holds both out^T (rows 0..63) and the softmax
    denominators (row 64), all computed on device.
Host side: transposes/scatters the band blocks into the zero [S, S]
matrix and divides by the device-computed denominators.
"""

import json

import numpy as np

import concourse.bass as bass
import concourse.bass2jax as bass2jax
import concourse.bass_utils as bass_utils
import concourse.mybir as mybir
import concourse.tile as tile
from concourse.bass import ts
from concourse.bass_utils import run_bass_kernel_spmd
from concourse.masks import make_identity

# ---------------------------------------------------------------------------
# Wait legalization: the walrus build in this container accepts only ONE
# sync wait per instruction (setupSyncWait: "Too many sync wait commands"),
# but Tile's semaphore assignment freely attaches several.  Split every
# excess wait into a standalone single-wait EventSemaphore instruction on
# the same engine immediately before the real instruction (the engine's
# sequencer blocks on each in turn — semantics preserved).
# ---------------------------------------------------------------------------

_MAX_WAITS = 1


def _legalize_block(bb, counter):
    out_instrs = []
    for ins in bb.get("instructions", []):
        si = ins.get("sync_info")
        waits = (si or {}).get("on_wait") or []
        if len(waits) > _MAX_WAITS:
            keep = waits[-_MAX_WAITS:]
            hoist = waits[:-_MAX_WAITS]
            for w in hoist:
                counter[0] += 1
                out_instrs.append({
                    "debug": ins.get("debug", 0),
                    "engine": ins["engine"],
                    "ins": [],
                    "name": f"evw_{counter[0]}_{ins['name']}",
                    "opcode": "EventSemaphore",
                    "outs": [],
                    "sync_info": {"on_update": [], "on_wait": [w]},
                })
            si["on_wait"] = keep
        out_instrs.append(ins)
    bb["instructions"] = out_instrs
    for sub in bb.get("blocks", []):
        _legalize_block(sub, counter)


def _legalize_waits(bir_json):
    d = json.loads(bir_json)
    counter = [0]
    for f in d.get("functions", []):
        for bb in f.get("blocks", []):
            _legalize_block(bb, counter)
    return json.dumps(d).encode()


_orig_compile_bir_kernel = bass_utils.compile_bir_kernel


def _patched_compile_bir_kernel(bir_json, tmpdir, neff_name="file.neff"):
    return _orig_compile_bir_kernel(_legalize_waits(bir_json), tmpdir,
                                    neff_name=neff_name)


if getattr(bass_utils.compile_bir_kernel, "__name__", "") != "_patched_compile_bir_kernel":
    bass_utils.compile_bir_kernel = _patched_compile_bir_kernel
    bass2jax.compile_bir_kernel = _patched_compile_bir_kernel


F32 = mybir.dt.float32
P = 128          # partition / query tile rows
D = 64           # head dim
DE = D + 1       # head dim + ones column (denominator row)
S = 2048         # sequence length
T = S // P       # 16 query tiles per head
W = 3 * P        # widest key window per query tile (2 prev blocks + diag)
NH = 4           # (b, h) pairs per core
NCORES = 8
TEMP = 8.0


def _build(repeat=1, _noband=False):
    nc = bass.Bass("TRN2", target_bir_lowering=False, debug=False)
    q = nc.dram_tensor("q", [NH, S, D], F32, kind="ExternalInput").ap()
    k = nc.dram_tensor("k", [NH, S, D], F32, kind="ExternalInput").ap()
    v = nc.dram_tensor("v", [NH, S, D], F32, kind="ExternalInput").ap()
    # out^T per tile with the denominator row: [hd, t, 65, q]
    out = nc.dram_tensor("out", [NH, T, DE, P], F32, kind="ExternalOutput").ap()
    # band[hd, t, kk, c*128+q] = P^T_unnorm block c of query tile t
    band = nc.dram_tensor("band", [NH, T, P, W], F32, kind="ExternalOutput").ap()

    with tile.TileContext(nc) as tc:
        with (
            tc.tile_pool(name="consts", bufs=1) as consts,
            tc.tile_pool(name="perhead", bufs=2) as perhead,
            tc.tile_pool(name="work", bufs=5) as work,
            tc.tile_pool(name="ps_tr", bufs=2, space="PSUM") as ps_tr,
            tc.tile_pool(name="ps_st", bufs=2, space="PSUM") as ps_st,
            tc.tile_pool(name="ps_o", bufs=2, space="PSUM") as ps_o,
        ):
            ident = consts.tile([P, P], F32)
            make_identity(nc, ident)

            # Multiplicative masks for S^T blocks [key jj (partition), q r]:
            # m2[:, 0, :]: diagonal block (j=0), allowed iff jj <= r.
            # m2[:, 1, :]: t-2 block (j=2),      allowed iff jj > r.
            m2 = consts.tile([P, 2, P], F32)
            nc.gpsimd.memset(m2, 1.0)
            nc.gpsimd.affine_select(
                m2[:, 0, :], m2[:, 0, :],
                compare_op=mybir.AluOpType.is_ge,
                fill=0.0, base=0, channel_multiplier=-1, pattern=[[1, P]],
            )
            nc.gpsimd.affine_select(
                m2[:, 1, :], m2[:, 1, :],
                compare_op=mybir.AluOpType.is_gt,
                fill=0.0, base=0, channel_multiplier=1, pattern=[[-1, P]],
            )

            def emit_head(hd):
                q_nat = perhead.tile([P, T, D], F32, tag="qnat")
                k_nat = perhead.tile([P, T, D], F32, tag="knat")
                v_sb = perhead.tile([P, T, DE], F32, tag="v")
                nc.sync.dma_start(q_nat, q[hd].rearrange("(t p) d -> p t d", p=P))
                nc.sync.dma_start(k_nat, k[hd].rearrange("(t p) d -> p t d", p=P))
                nc.sync.dma_start(v_sb[:, :, 0:D],
                                  v[hd].rearrange("(t p) d -> p t d", p=P))
                nc.gpsimd.memset(v_sb[:, :, D:DE], 1.0)

                # q, k transposed to [D, t, 128] (contraction dim on partitions)
                qT = perhead.tile([D, T, P], F32, tag="qT")
                kT = perhead.tile([D, T, P], F32, tag="kT")
                for t4 in range(T // 4):
                    tq = ps_tr.tile([D, 4, P], F32, tag="tqk")
                    tk = ps_tr.tile([D, 4, P], F32, tag="tqk")
                    for j in range(4):
                        t = 4 * t4 + j
                        nc.tensor.transpose(tq[:, j], q_nat[:, t, :], ident)
                        nc.tensor.transpose(tk[:, j], k_nat[:, t, :], ident)
                    nc.scalar.copy(qT[:, ts(t4, 4), :], tq)
                    nc.vector.tensor_copy(kT[:, ts(t4, 4), :], tk)

                oT_all = perhead.tile([DE, T, P], F32, tag="o")

                # Iteration c produces P^T block c (vs query tiles c..c+2).
                # P @ V for tile t runs one iteration LATE (t = c - 1) so the
                # PE never waits on the same iteration's exp+mask chain.
                LAG = 2
                o4 = [None]
                pTb = [None] * T

                def emit_pv(t):
                    w0b = max(0, t - 2)
                    if t % 4 == 0:
                        o4[0] = ps_o.tile([DE, 4, P], F32, tag="o4", name="o4")
                    for c2 in range(w0b, t + 1):
                        nc.tensor.matmul(
                            o4[0][:, t % 4, :], v_sb[:, c2, :],
                            pTb[c2][:, t - c2, :],
                            start=(c2 == w0b), stop=(c2 == t),
                        )
                    if t % 4 == 3:
                        nc.vector.tensor_copy(oT_all[:, t - 3:t + 1, :], o4[0])

                for c in range(T):
                    jn = min(3, T - c)      # query tiles covered by block c

                    stp = ps_st.tile([P, 3, P], F32, tag="st")
                    nc.tensor.matmul(
                        stp[:, :jn, :], kT[:, c, :], qT[:, c:c + jn, :],
                        start=True, stop=True,
                    )

                    pT = work.tile([P, 3, P], F32, tag="pT")
                    pTb[c] = pT
                    nc.scalar.activation(
                        pT[:, :jn, :], stp[:, :jn, :],
                        mybir.ActivationFunctionType.Exp, scale=1.0 / TEMP,
                    )
                    # band masks (exact 0 for disallowed entries): j=0 is the
                    # diagonal block of tile c, j=2 the t-2 block of tile c+2
                    if jn == 3:
                        nc.gpsimd.tensor_mul(pT[:, 0::2, :], pT[:, 0::2, :], m2)
                    else:
                        nc.gpsimd.tensor_mul(pT[:, 0, :], pT[:, 0, :],
                                             m2[:, 0, :])

                    if not _noband:
                        nc.sync.dma_start(band[hd, c, :, :jn * P], pT[:, :jn, :])

                    if c - LAG >= 0:
                        emit_pv(c - LAG)
                for t in range(T - LAG, T):
                    emit_pv(t)

                nc.sync.dma_start(out[hd].rearrange("t e q -> e t q"), oT_all)

            def emit_heads():
                for hd in range(NH):
                    emit_head(hd)

            if repeat == 1:
                emit_heads()
            else:
                with tc.For_i(0, repeat, 1):
                    emit_heads()
    return nc


LAST_RESULTS = None


def _run(in_maps, trace=False):
    global LAST_RESULTS
    nc = _build()
    LAST_RESULTS = run_bass_kernel_spmd(
        nc, in_maps, core_ids=list(range(NCORES)), trace=trace,
    )
    return LAST_RESULTS.results


def kernel(q, k, v, _trace=False):
    q = np.ascontiguousarray(np.asarray(q, dtype=np.float32))
    k = np.ascontiguousarray(np.asarray(k, dtype=np.float32))
    v = np.ascontiguousarray(np.asarray(v, dtype=np.float32))
    B, H, S_, D_ = q.shape
    assert (S_, D_) == (S, D), (S_, D_)
    G = B * H
    per = G // NCORES
    assert per == NH

    qf = q.reshape(G, S, D)
    kf = k.reshape(G, S, D)
    vf = v.reshape(G, S, D)
    in_maps = [
        {
            "q": np.ascontiguousarray(qf[i * per:(i + 1) * per]),
            "k": np.ascontiguousarray(kf[i * per:(i + 1) * per]),
            "v": np.ascontiguousarray(vf[i * per:(i + 1) * per]),
        }
        for i in range(NCORES)
    ]

    results = _run(in_maps, trace=_trace)

    out = np.empty((G, S, D), np.float32)
    attn = np.zeros((G, S, S), np.float32)
    for i in range(NCORES):
        outr = results[i]["out"]                       # [NH, T, DE, P]
        bandr = results[i]["band"]                     # [NH, C=T, P, W]
        for g in range(per):
            gi = i * per + g
            den = outr[g, :, D, :]                     # [T, P] per-query sums
            ot = outr[g, :, :D, :] / den[:, None, :]   # [T, D, P]
            out[gi] = ot.transpose(0, 2, 1).reshape(S, D)
            # band[g, c, kk, j*128+q] -> attn[128(c+j)+q, 128c+kk]
            bt = bandr[g].reshape(T, P, 3, P)          # [c, kk, j, q]
            bt = bt.transpose(0, 2, 3, 1)              # [c, j, q, kk]
            for c in range(T):
                for j in range(min(3, T - c)):
                    t = c + j
                    attn[gi, t * P:(t + 1) * P, c * P:(c + 1) * P] = (
                        bt[c, j] / den[t][:, None]
                    )
    return out.reshape(B, H, S, D), attn.reshape(B, H, S, S)
